# revision 71
# baseline (speedup 1.0000x reference)
"""Trainium2 Bass kernel for batched dot-product attention.

Problem: q, kv [B=4, H=8, S=2048, D=64] fp32, mask [1, 1, S, S] fp32.
    out = softmax(q @ kv^T / sqrt(D) + mask) @ kv

Sharding: the 32 (b, h) pairs are split across 8 NeuronCores, 4 pairs
per core. Each core computes its pairs' full S x S attention locally;
no cross-device communication.

Per-pair device algorithm (fast path, mask == 0):
  1. q, kv are DMA'd in fp32 with fat per-partition descriptors (row r
     at partition r//16), cast to bf16 (GpSimdE steady-state / VectorE
     for the latency-critical pair 0), staged to a DRAM scratch
     [S, 128] with the 64 columns DUPLICATED into both halves via two
     small-descriptor DMA writes (XBAR transpose needs a 2-byte dtype;
     fat single-partition descriptors would serialize on one SBUF
     port), and DMA-transposed back into qT/kvT [128, S] bf16 tiles
     holding the transposed tensor in BOTH partition ranges 0-63 and
     64-127. kv's sk block k is the STRIDED row set {c*16+k} - mm1
     (via scratch row order) and mm2 (via kv_aug's o index) agree, and
     softmax + the mm2 reduction are order-agnostic in sk. The pair-0
     chain is chunked x2 and split across the Sync and Scalar HWDGE
     queues so the first matmul can start at ~13us.
  2. scoreT[sk, sq] = kvT.T @ qT per 128-row sk block into PSUM: the
     duplicated halves let two K=64 matmuls (sk blocks 2i, 2i+1) run
     CONCURRENTLY in the two PE row-group halves. The exp drain is
     SPLIT between two engines, tile by tile:
       - ScalarE: activation exp(0.125 * scoreT) -> bf16 attnT.
       - VectorE: a Schraudolph-style one-instruction exp. tensor_scalar
         computes w = scoreT*C1 + C2 in fp32 and converts to uint16 with
         round-to-nearest; C1/C2 are chosen so that integer IS the bf16
         bit pattern of exp(0.125*scoreT) (max mult. error +-3.4%,
         which washes to ~4e-3 rel error after softmax normalization).
     Splitting 19:13 keeps both engines' exp streams at ~83us, below
     the PE's ~128us, so the kernel is TensorE-bound (~90% PE busy).
     Softmax max-subtraction is skipped: scores are ~N(0,64) pre-scale
     so exp is safe in fp32, matching the reference to ~4e-3.
  3. outT[d, sq] (+ a denominator row) = kv_aug.T @ attnT accumulated
     over the 16 sk blocks, where kv_aug [128, 16, 65] bf16 is kv with
     a ones column: row 64 of outT is the softmax denominator. The cast
     writes kv's bf16 directly into kv_aug's strided columns.
  4. outT 128-column blocks are transposed back on TensorE (identity
     matmul) in groups of 4 into one PSUM tile [128, 4, 65]; one
     VectorE reciprocal + one broadcast multiply normalize the group,
     and one DMA stores 512 rows of fp32 output.

mm1 iterates half-outer (all 16 sk blocks for sq cols [0,1024) before
cols [1024,2048)), so each pair's mm2 n-tiles 0-1 start while mm1 still
runs on the second half, shrinking the pipeline tail. A warmup burst of
junk matmuls during the prologue DMAs keeps the PE HAM clock-gate
released (1.2 -> 2.4 GHz) before the first real matmul; pair-0's early
slots add score-slice fillers so the exp-paced ramp never idles the PE
long enough to re-engage the gate. Engine-queue discipline matters
everywhere: an instruction that waits at the head of an in-order queue
(HWDGE Sync/Scalar, or the PE) blocks everything behind it, so work is
emitted so that queue order matches data-readiness order.

If mask is nonzero (never the case for this problem's setup_inputs,
which zero-fills it), a variant NEFF streams mask^T tiles and adds them
to scoreT before a ScalarE-only exp. Slower, but correct.
"""

import math

import numpy as np

B, H, S, D = 4, 8, 2048, 64
N_CORES = 8
NP = (B * H) // N_CORES  # pairs per core = 4
P = 128
SK_BLKS = S // P   # 16
NT = S // 512      # 4 sq tiles of 512
KCOLS = D + 1      # kv columns + ones column
HB = 1024          # score tile free size (2 PSUM banks)
WARMUP = 26        # prologue junk matmuls (HAM clock release)

# Schraudolph constants: uint16(round(score*C1 + C2)) == bf16 bits of
# ~exp(score/8).  C1 = 128*log2(e)/8, C2 = 128*(127 - c_opt).
SCH_C1 = float(16.0 * math.log2(math.e))
SCH_C2 = float(128.0 * (127.0 - 0.0380))

# Of the 32 exp tiles per pair, this many go to VectorE (Bresenham-
# interleaved with ScalarE tiles so the two PSUM drains overlap).
DVE_N = 13         # of 32 exp tiles per pair, this many go to VectorE
_DVE_SLOTS = frozenset(
    s for s in range(32) if (s * DVE_N) // 32 != ((s + 1) * DVE_N) // 32)


def _install_wait_split():
    """Split multi-sem-wait instructions into single-wait NoOp carriers.

    The walrus build in this container rejects any instruction whose
    sync_info.on_wait has more than one entry ("Too many sync wait
    commands"). Engines execute their stream in order, so hoisting all
    but one wait onto same-engine NoOps directly before the instruction
    is semantically identical.
    """
    import orjson
    import concourse.bass2jax as bass2jax
    import concourse.bass_utils as bass_utils

    if getattr(bass2jax.compile_bir_kernel, "_wait_split", False):
        return

    def split_multi_waits(bir_json):
        d = orjson.loads(bir_json)
        for fn in d.get("functions", []):
            for blk in fn.get("blocks", []):
                out = []
                for inst in blk.get("instructions", []):
                    si = inst.get("sync_info") or {}
                    ow = si.get("on_wait") or []
                    if len(ow) > 1:
                        for j, w in enumerate(ow[:-1]):
                            out.append({
                                "engine": inst["engine"],
                                "ins": [],
                                "name": f"{inst['name']}-w{j}",
                                "opcode": "NoOp",
                                "outs": [],
                                "sync_info": {"on_wait": [w]},
                            })
                        si["on_wait"] = [ow[-1]]
                    out.append(inst)
                blk["instructions"] = out
        return orjson.dumps(d)

    orig = bass_utils.compile_bir_kernel

    def patched(bir_json, tmpdir, neff_name="file.neff"):
        return orig(split_multi_waits(bir_json), tmpdir, neff_name=neff_name)

    patched._wait_split = True
    bass2jax.compile_bir_kernel = patched


def _install_ntff_hook():
    """Register the ctypes NTFF profile hook missing from this image's
    antenv, so run_bass_kernel_spmd(trace=True) can report exec time."""
    import contextlib
    import ctypes
    import sys
    import types

    if "antenv.axon_hooks" in sys.modules:
        return

    so_path = "/opt/axon/libaxon_pjrt.so"
    try:
        lib = ctypes.CDLL(so_path)
    except OSError:
        return
    if not hasattr(lib, "axon_start_nrt_profile"):
        return
    lib.axon_start_nrt_profile.argtypes = [ctypes.POINTER(ctypes.c_int64),
                                           ctypes.c_size_t]
    lib.axon_start_nrt_profile.restype = ctypes.c_int64
    lib.axon_stop_nrt_profile.argtypes = [ctypes.c_char_p]
    lib.axon_stop_nrt_profile.restype = ctypes.c_int64

    @contextlib.contextmanager
    def _hook(output_dir, device_ids):
        import jax
        jax.devices()
        if device_ids:
            ids = (ctypes.c_int64 * len(device_ids))(*device_ids)
            rc = lib.axon_start_nrt_profile(ids, len(device_ids))
        else:
            rc = lib.axon_start_nrt_profile(None, 0)
        if rc != 0:
            raise RuntimeError(f"axon_start_nrt_profile rc={rc}")
        try:
            yield
        finally:
            n = lib.axon_stop_nrt_profile(str(output_dir).encode())
            print(f"ntff profile: {n} file(s) in {output_dir}", file=sys.stderr)

    mod = types.ModuleType("antenv.axon_hooks")
    mod.get_axon_ntff_profile_hook = lambda: _hook
    mod.set_axon_ntff_profile_hook = lambda h: None
    sys.modules["antenv.axon_hooks"] = mod
    import antenv
    antenv.axon_hooks = mod


_module_cache = {}


def _build_module(with_mask):
    import concourse.bass as bass
    import concourse.mybir as mybir
    import concourse.tile as tile
    from concourse.masks import make_identity
    from collections import deque
    from contextlib import ExitStack

    f32 = mybir.dt.float32
    bf16 = mybir.dt.bfloat16
    u16 = mybir.dt.uint16
    Exp = mybir.ActivationFunctionType.Exp

    nc = bass.Bass("TRN2", target_bir_lowering=False)
    q_s = nc.dram_tensor("q_s", [NP, S, D], f32, kind="ExternalInput")
    kv_s = nc.dram_tensor("kv_s", [NP, S, D], f32, kind="ExternalInput")
    out_s = nc.dram_tensor("out_s", [NP, S, D], f32, kind="ExternalOutput")
    mask_t = None
    if with_mask:
        mask_t = nc.dram_tensor("mask_t", [S, S], f32, kind="ExternalInput")

    with tile.TileContext(nc) as tc, ExitStack() as ctx:
        io = ctx.enter_context(tc.tile_pool(name="io", bufs=2))
        kvp = ctx.enter_context(tc.tile_pool(name="kvp", bufs=3))
        tduo = ctx.enter_context(tc.tile_pool(name="tduo", bufs=2))
        big = ctx.enter_context(tc.tile_pool(name="big", bufs=2))
        outp = ctx.enter_context(tc.tile_pool(name="outp", bufs=2))
        res = ctx.enter_context(tc.tile_pool(name="res", bufs=3))
        cons = ctx.enter_context(tc.tile_pool(name="cons", bufs=1))
        dram = ctx.enter_context(tc.tile_pool(name="dram", bufs=2, space="DRAM"))
        # PSUM budget (8 banks): 3 x [128, 1024] score tiles (6 banks,
        # triple-buffered so the PE runs 1.5 mm1 calls ahead of the exp
        # drain - anything less re-engages the HAM clock gate) + a
        # 2-slot pool (1 bank each) shared by the mm2 accumulator and
        # the finalize transpose groups.
        ps_score = ctx.enter_context(tc.tile_pool(name="ps_score", bufs=3, space="PSUM"))
        ps_mask = (ctx.enter_context(tc.tile_pool(name="ps_mask", bufs=2))
                   if with_mask else None)
        ps_small = ctx.enter_context(tc.tile_pool(name="ps_small", bufs=2, space="PSUM"))

        identity = cons.tile([KCOLS, KCOLS], f32, tag="identity", name="identity")
        make_identity(nc, identity)

        # Warmup burst: junk matmuls queued while the prologue DMAs are
        # in flight keep the PE array busy so the HAM clock-gate
        # releases (1.2 -> 2.4 GHz) before the first real matmul.
        junk = cons.tile([P, 512], bf16, tag="junk", name="junk")
        nc.vector.memset(junk[:], 0.5)
        wtile = ps_small.tile([KCOLS, 512], f32, tag="pst", name="warm")
        for _ in range(WARMUP):
            nc.tensor.matmul(wtile[:, 0:512][:KCOLS], lhsT=junk[:, 0:KCOLS],
                             rhs=junk[:], start=True, stop=True)

        state = [dict() for _ in range(NP)]

        def prep_alloc(p):
            qf = io.tile([P, SK_BLKS, D], f32, tag="qf", name="qf")
            kf = io.tile([P, SK_BLKS, D], f32, tag="kf", name="kf")
            qb = io.tile([P, SK_BLKS, D], bf16, tag="qb", name="qb")
            kv_aug = kvp.tile([P, SK_BLKS, KCOLS], bf16, tag="kv_aug", name="kv_aug")
            qT = tduo.tile([P, S], bf16, tag="qT", name="qT")
            # kvT columns hold sk block j (the strided row set
            # {cc*16+j}, consistent with kv_aug's o index) at
            # [128j, 128j+128): mm1 reads contiguous column blocks.
            kvT = tduo.tile([P, S], bf16, tag="kvT", name="kvT")
            scr_q = dram.tile([S, P], bf16, tag="scr_q", name="scr_q")
            scr_kv = dram.tile([S, P], bf16, tag="scr_kv", name="scr_kv")
            state[p].update(qf=qf, kf=kf, qb=qb, kv_aug=kv_aug,
                            qT=qT, kvT=kvT, scr_q=scr_q, scr_kv=scr_kv)

        def prep_ins(p, dma_q=None, dma_kv=None):
            # Inbound fp32 DMAs. Row r of q/kv lands at partition r//16,
            # free index r%16: 16 consecutive rows = 4KB contiguous per
            # partition -> 128 fat descriptors, ~1.5us. For kv this
            # redefines sk block k as the STRIDED row set {c*16+k}; mm1
            # (via scr_kv's row order) and mm2 (via kv_aug's o index)
            # agree on the decomposition, and softmax + the mm2
            # reduction are order-agnostic in sk. q keeps the identity
            # row order (sq indexes the output).
            st = state[p]
            (dma_q or nc.sync).dma_start(
                st["qf"][:], q_s[p].rearrange("(pp o) d -> pp o d", o=SK_BLKS))
            (dma_kv or nc.sync).dma_start(
                st["kf"][:], kv_s[p].rearrange("(pp o) d -> pp o d", o=SK_BLKS))

        def prep_cast(p, q_cast, kv_cast):
            # kv's bf16 goes straight into kv_aug's strided columns;
            # the scratch writes read from there.
            st = state[p]
            kv_cast.tensor_copy(out=st["kv_aug"][:, :, 0:D], in_=st["kf"][:])
            kv_cast.memset(st["kv_aug"][:, :, D:KCOLS], 1.0)
            q_cast.tensor_copy(out=st["qb"][:], in_=st["qf"][:])

        def prep_scr(p, which, c, dma=None):
            # Stage one tensor: the bf16 copy is written TWICE into the
            # row-duplicated [2, S, 64] DRAM scratch - both writes are
            # fat 2KB-per-partition descriptors - then each copy is
            # XBAR-transposed (2-byte dtype) into one 64-partition half
            # of qT/kvT, so mm1 runs two k-steps concurrently in the two
            # PE row-group halves. Scratch row r holds q/kv row r in
            # (pp o) order: identity for q (sq order preserved); for kv
            # the transposed column order cc*16+j IS the strided block
            # decomposition that kv_aug/mm2 use. A transpose follows its
            # own scratch writes on the SAME queue, so it never
            # head-blocks another chain stage.
            st = state[p]
            dma = dma or nc.sync
            if which == "q":
                # scr_q row pp*16+o <- qb[pp, o]: identity row order;
                # chunk c = partitions [64c, 64c+64) = scratch rows /
                # qT cols [1024c, +1024). Two writes (the duplicated
                # 64-col halves) keep descriptors small - DMA reads of
                # one SBUF partition serialize, so fatter is SLOWER.
                scr_v = st["scr_q"].rearrange(
                    "(pp o) (u dd) -> pp o u dd", o=SK_BLKS, dd=D)
                for u in (0, 1):
                    dma.dma_start(scr_v[64 * c:64 * c + 64, :, u, :],
                                  st["qb"][64 * c:64 * c + 64, :, :])
                dma.dma_start_transpose(st["qT"][:, c * 1024:(c + 1) * 1024],
                                        st["scr_q"][c * 1024:(c + 1) * 1024, :])
            else:
                # scr_kv row j*128+cc <- kv_aug[cc, j] = kv row cc*16+j:
                # sk block j (the strided row set {cc*16+j}) lands in
                # contiguous scratch rows [128j, 128j+128) = the kvT
                # column block mm1 reads; chunk c = blocks [8c, 8c+8).
                scr_v = st["scr_kv"].rearrange(
                    "(o pp) (u dd) -> pp o u dd", pp=P, dd=D)
                for u in (0, 1):
                    dma.dma_start(scr_v[:, 8 * c:8 * c + 8, u, :],
                                  st["kv_aug"][:, 8 * c:8 * c + 8, 0:D])
                dma.dma_start_transpose(st["kvT"][:, c * 1024:(c + 1) * 1024],
                                        st["scr_kv"][c * 1024:(c + 1) * 1024, :])

        def mm1_half(p, ip, half, slot, fillers=0):
            # scoreT [128 sk x 1024 sq] for TWO sk blocks 2*ip and 2*ip+1,
            # run concurrently in PE row groups 0-63 / 64-127.
            st = state[p]
            scs = []
            for mb in (0, 1):
                i = 2 * ip + mb
                h0 = D * mb
                sc = ps_score.tile([P, HB], f32, tag="sc", name="sc")
                scs.append((i, h0, sc))
            for _ in range(fillers):
                for (i, h0, sc) in scs:
                    nc.tensor.matmul(
                        sc[:, 0:512],
                        lhsT=st["kvT"][h0:h0 + D, i * P:(i + 1) * P],
                        rhs=st["qT"][h0:h0 + D, 0:512],
                        start=True, stop=True)
            # n-major so adjacent instructions target the two PE row
            # groups and execute concurrently (mb-major serializes: the
            # in-order dispatch blocks mb1 behind mb0's second matmul).
            for n in range(HB // 512):
                c0 = half * HB + n * 512
                for (i, h0, sc) in scs:
                    nc.tensor.matmul(
                        sc[:, n * 512:(n + 1) * 512],
                        lhsT=st["kvT"][h0:h0 + D, i * P:(i + 1) * P],
                        rhs=st["qT"][h0:h0 + D, c0:c0 + 512],
                        start=True, stop=True)
            for idx, (i, h0, sc) in enumerate(scs):
                at = st["attnT"][:, i, half * HB:(half + 1) * HB]
                if with_mask:
                    mt = ps_mask.tile([P, HB], f32, tag="mt", name="mt")
                    nc.sync.dma_start(mt[:], mask_t[i * P:(i + 1) * P,
                                                    half * HB:(half + 1) * HB])
                    nc.vector.scalar_tensor_tensor(
                        out=sc[:], in0=sc[:], scalar=0.125, in1=mt[:],
                        op0=mybir.AluOpType.mult, op1=mybir.AluOpType.add)
                    nc.scalar.activation(at, sc[:], Exp)
                elif (2 * slot + idx) in _DVE_SLOTS:
                    # One-instruction exp: fp32 w = sc*C1 + C2 converts
                    # to uint16 (round-to-nearest) == bf16 exp bits.
                    nc.vector.tensor_scalar(
                        out=at.bitcast(u16), in0=sc[:],
                        scalar1=SCH_C1, scalar2=SCH_C2,
                        op0=mybir.AluOpType.mult, op1=mybir.AluOpType.add)
                else:
                    # exp((q @ kv^T) * 0.125): the 1/sqrt(D) folds into
                    # the activation's free affine scale.
                    nc.scalar.activation(at, sc[:], Exp, scale=0.125)

        def mm2_subchunk(p, n, k0, po):
            # Continue outT[0:65, n*512:(n+1)*512] over sk blocks k0..k0+3.
            st = state[p]
            for k in range(k0, k0 + 4):
                nc.tensor.matmul(
                    po[:],
                    lhsT=st["kv_aug"][:, k, :],
                    rhs=st["attnT"][:, k, n * 512:(n + 1) * 512],
                    start=(k == 0), stop=(k == SK_BLKS - 1))
            if k0 + 4 == SK_BLKS:
                nc.vector.tensor_copy(
                    out=st["outT"][:, n * 512:(n + 1) * 512], in_=po[:])

        def finalize_g(p, g):
            # Transpose 128-column blocks 4g..4g+3 back to [sq, d] in one
            # PSUM group (borrowing a score-ring slot), normalize with
            # one reciprocal + one broadcast multiply, store 512 rows
            # with one DMA.
            st = state[p]
            tp4 = ps_small.tile([P, 4, KCOLS], f32, tag="pst", name="tp4")
            for jj in range(4):
                j = 4 * g + jj
                nc.tensor.transpose(tp4[:, jj, :],
                                    st["outT"][:, j * P:(j + 1) * P], identity[:])
            rec4 = res.tile([P, 4], f32, tag="rec4", name="rec4")
            nc.vector.reciprocal(rec4[:], tp4[:, :, D])
            ob4 = res.tile([P, 4, D], f32, tag="ob4", name="ob4")
            nc.vector.tensor_tensor(
                out=ob4[:], in0=tp4[:, :, 0:D],
                in1=rec4[:, :, None].broadcast_to([P, 4, D]),
                op=mybir.AluOpType.mult)
            nc.sync.dma_start(
                out_s[p, g * 512:(g + 1) * 512, :].rearrange(
                    "(j pp) d -> pp j d", pp=P),
                ob4[:])

        sub_q = deque()    # (pair, n, k0)
        fins_q = deque()   # (pair, g)
        chunks_done = [0] * NP
        cur_po = [None]

        def pop_sub():
            if sub_q:
                p, n, k0 = sub_q.popleft()
                if k0 == 0:
                    cur_po[0] = ps_small.tile([KCOLS, 512], f32, tag="pst", name="po")
                mm2_subchunk(p, n, k0, cur_po[0])
                if k0 + 4 == SK_BLKS:
                    chunks_done[p] += 1

        def pop_fin():
            if fins_q:
                p, g = fins_q[0]
                if g < chunks_done[p]:
                    fins_q.popleft()
                    finalize_g(p, g)

        # Pair 0's prep is chunked x4 so the first mm1 can start as soon
        # as the first chunk's transposes land instead of waiting for
        # the full chain.
        # Pair 0's prep is latency-critical: q's whole chain rides the
        # otherwise-idle Scalar HWDGE queue in parallel with kv's on
        # Sync, and casts go to VectorE (idle in the prologue, 4x
        # faster than GpSimdE).
        prep_alloc(0)
        prep_ins(0, dma_q=nc.scalar, dma_kv=nc.sync)
        prep_cast(0, q_cast=nc.vector, kv_cast=nc.vector)
        # First-needed chunks split across the queues; each queue then
        # carries one second chunk, so everything lands by ~slot 4.
        prep_scr(0, "q", 0, dma=nc.scalar)
        prep_scr(0, "k", 0, dma=nc.sync)
        prep_scr(0, "k", 1, dma=nc.scalar)
        prep_scr(0, "q", 1, dma=nc.sync)
        for p in range(NP):
            state[p]["attnT"] = big.tile([P, SK_BLKS, S], bf16, tag="attnT", name="attnT")
            state[p]["outT"] = outp.tile([KCOLS, S], f32, tag="outT", name="outT")
            slot = 0
            for half in range(S // HB):
                for ip in range(SK_BLKS // 2):
                    # Emit the independent backlog first so the PE stream
                    # never has a dependent mm1 at its head while older
                    # work could run. Fins pop before subs so a fin's
                    # transposes trail its outT copies by a full slot.
                    pop_fin()
                    pop_sub()
                    # Pair 0's early slots have no mm2/fin backlog to
                    # absorb the exp-paced ramp; fillers writing the
                    # about-to-be-overwritten score slices keep the PE
                    # dense so the HAM clock-gate stays released.
                    mm1_half(p, ip, half, slot,
                             fillers=2 if p == 0 and slot < 6 else 0)
                    slot += 1
                    # Next pair's prep, emitted early with casts on the
                    # idle GpSimdE (kv first: mm1 walks all kv blocks
                    # within its first half) and all DMAs on Sync, so
                    # the ~12us chain lands well before that pair's mm1.
                    if p + 1 < NP:
                        if slot == 1:
                            prep_alloc(p + 1)
                            prep_ins(p + 1)
                            prep_cast(p + 1, q_cast=nc.gpsimd,
                                      kv_cast=nc.gpsimd)
                        elif slot == 3:
                            prep_scr(p + 1, "k", 0)
                            prep_scr(p + 1, "q", 0)
                        elif slot == 5:
                            prep_scr(p + 1, "k", 1)
                            prep_scr(p + 1, "q", 1)
                # This half's sq cols now have all 16 sk blocks: queue mm2.
                for n in (2 * half, 2 * half + 1):
                    for k0 in range(0, SK_BLKS, 4):
                        sub_q.append((p, n, k0))
                fins_q.append((p, 2 * half))
                fins_q.append((p, 2 * half + 1))
        while sub_q or fins_q:
            pop_sub()
            pop_fin()

    return nc


def _get_module(with_mask):
    if with_mask not in _module_cache:
        _install_wait_split()
        _install_ntff_hook()
        _module_cache[with_mask] = _build_module(with_mask)
    return _module_cache[with_mask]


def _run(q, kv, mask, trace=False, tmpdir=None):
    from concourse.bass_utils import run_bass_kernel_spmd

    q = np.ascontiguousarray(np.asarray(q), dtype=np.float32)
    kv = np.ascontiguousarray(np.asarray(kv), dtype=np.float32)
    mask = np.asarray(mask)
    with_mask = bool(np.any(mask))

    nc = _get_module(with_mask)

    qf = q.reshape(B * H, S, D)
    kf = kv.reshape(B * H, S, D)
    in_maps = []
    for c in range(N_CORES):
        m = {
            "q_s": np.ascontiguousarray(qf[c * NP:(c + 1) * NP]),
            "kv_s": np.ascontiguousarray(kf[c * NP:(c + 1) * NP]),
        }
        if with_mask:
            m["mask_t"] = np.ascontiguousarray(
                mask.reshape(S, S).T, dtype=np.float32)
        in_maps.append(m)

    kw = {}
    if trace:
        kw = dict(trace=True, tmpdir=tmpdir)
    bres = run_bass_kernel_spmd(nc, in_maps, core_ids=list(range(N_CORES)), **kw)
    out = np.stack([bres.results[c]["out_s"] for c in range(N_CORES)])
    out = out.reshape(B, H, S, D).astype(np.float32, copy=False)
    return out, bres


def kernel(q, kv, mask):
    out, _ = _run(q, kv, mask)
    return out


# revision 72
# speedup vs baseline: 1.0952x; 1.0952x over previous
"""Trainium2 Bass kernel for batched dot-product attention.

Problem: q, kv [B=4, H=8, S=2048, D=64] fp32, mask [1, 1, S, S] fp32.
    out = softmax(q @ kv^T / sqrt(D) + mask) @ kv

Sharding: the 32 (b, h) pairs are split across 8 NeuronCores, 4 pairs
per core. Each core computes its pairs' full S x S attention locally;
no cross-device communication.

Per-pair device algorithm (fast path, mask == 0):
  1. q, kv are DMA'd in fp32 with fat per-partition descriptors (row r
     at partition r//16), cast to bf16 (GpSimdE steady-state / VectorE
     for the latency-critical pair 0), staged to a DRAM scratch
     [S, 128] with the 64 columns DUPLICATED into both halves via two
     small-descriptor DMA writes (XBAR transpose needs a 2-byte dtype;
     fat single-partition descriptors would serialize on one SBUF
     port), and DMA-transposed back into qT/kvT [128, S] bf16 tiles
     holding the transposed tensor in BOTH partition ranges 0-63 and
     64-127. kv's sk block k is the STRIDED row set {c*16+k} - mm1
     (via scratch row order) and mm2 (via kv_aug's o index) agree, and
     softmax + the mm2 reduction are order-agnostic in sk. The pair-0
     chain is chunked x2 and split across the Sync and Scalar HWDGE
     queues so the first matmul can start at ~13us.
  2. scoreT[sk, sq] = kvT.T @ qT per 128-row sk block into PSUM: the
     duplicated halves let two K=64 matmuls (sk blocks 2i, 2i+1) run
     CONCURRENTLY in the two PE row-group halves. The exp drain is
     SPLIT between two engines, tile by tile:
       - ScalarE: activation exp(0.125 * scoreT) -> bf16 attnT.
       - VectorE: a Schraudolph-style one-instruction exp. tensor_scalar
         computes w = scoreT*C1 + C2 in fp32 and converts to uint16 with
         round-to-nearest; C1/C2 are chosen so that integer IS the bf16
         bit pattern of exp(0.125*scoreT) (max mult. error +-3.4%,
         which washes to ~4e-3 rel error after softmax normalization).
     Splitting 19:13 keeps both engines' exp streams at ~83us, below
     the PE's ~128us, so the kernel is TensorE-bound (~90% PE busy).
     Softmax max-subtraction is skipped: scores are ~N(0,64) pre-scale
     so exp is safe in fp32, matching the reference to ~4e-3.
  3. outT[d, sq] (+ a denominator row) = kv_aug.T @ attnT accumulated
     over the 16 sk blocks, where kv_aug [128, 16, 65] bf16 is kv with
     a ones column: row 64 of outT is the softmax denominator. The cast
     writes kv's bf16 directly into kv_aug's strided columns.
  4. outT 128-column blocks are transposed back on TensorE (identity
     matmul) in groups of 4 into one PSUM tile [128, 4, 65]; one
     VectorE reciprocal + one broadcast multiply normalize the group,
     and one DMA stores 512 rows of fp32 output.

mm1 iterates half-outer (all 16 sk blocks for sq cols [0,1024) before
cols [1024,2048)), so each pair's mm2 n-tiles 0-1 start while mm1 still
runs on the second half, shrinking the pipeline tail. A warmup burst of
junk matmuls during the prologue DMAs keeps the PE HAM clock-gate
released (1.2 -> 2.4 GHz) before the first real matmul; pair-0's early
slots add score-slice fillers so the exp-paced ramp never idles the PE
long enough to re-engage the gate. Engine-queue discipline matters
everywhere: an instruction that waits at the head of an in-order queue
(HWDGE Sync/Scalar, or the PE) blocks everything behind it, so work is
emitted so that queue order matches data-readiness order.

If mask is nonzero (never the case for this problem's setup_inputs,
which zero-fills it), a variant NEFF streams mask^T tiles and adds them
to scoreT before a ScalarE-only exp. Slower, but correct.
"""

import math

import numpy as np

B, H, S, D = 4, 8, 2048, 64
N_CORES = 8
NP = (B * H) // N_CORES  # pairs per core = 4
P = 128
SK_BLKS = S // P   # 16
NT = S // 512      # 4 sq tiles of 512
KCOLS = D + 1      # kv columns + ones column
HB = 1024          # score tile free size (2 PSUM banks)
WARMUP = 14        # prologue junk matmuls (HAM clock release)

# Schraudolph constants: uint16(round(score*C1 + C2)) == bf16 bits of
# ~exp(score/8).  C1 = 128*log2(e)/8, C2 = 128*(127 - c_opt).
SCH_C1 = float(16.0 * math.log2(math.e))
SCH_C2 = float(128.0 * (127.0 - 0.0380))

# Of the 32 exp tiles per pair, this many go to VectorE (Bresenham-
# interleaved with ScalarE tiles so the two PSUM drains overlap).
DVE_N = 13         # of 32 exp tiles per pair, this many go to VectorE
_DVE_SLOTS = frozenset(
    s for s in range(32) if (s * DVE_N) // 32 != ((s + 1) * DVE_N) // 32)


def _install_wait_split():
    """Split multi-sem-wait instructions into single-wait NoOp carriers.

    The walrus build in this container rejects any instruction whose
    sync_info.on_wait has more than one entry ("Too many sync wait
    commands"). Engines execute their stream in order, so hoisting all
    but one wait onto same-engine NoOps directly before the instruction
    is semantically identical.
    """
    import orjson
    import concourse.bass2jax as bass2jax
    import concourse.bass_utils as bass_utils

    if getattr(bass2jax.compile_bir_kernel, "_wait_split", False):
        return

    def split_multi_waits(bir_json):
        d = orjson.loads(bir_json)
        for fn in d.get("functions", []):
            for blk in fn.get("blocks", []):
                out = []
                for inst in blk.get("instructions", []):
                    si = inst.get("sync_info") or {}
                    ow = si.get("on_wait") or []
                    if len(ow) > 1:
                        for j, w in enumerate(ow[:-1]):
                            out.append({
                                "engine": inst["engine"],
                                "ins": [],
                                "name": f"{inst['name']}-w{j}",
                                "opcode": "NoOp",
                                "outs": [],
                                "sync_info": {"on_wait": [w]},
                            })
                        si["on_wait"] = [ow[-1]]
                    out.append(inst)
                blk["instructions"] = out
        return orjson.dumps(d)

    orig = bass_utils.compile_bir_kernel

    def patched(bir_json, tmpdir, neff_name="file.neff"):
        return orig(split_multi_waits(bir_json), tmpdir, neff_name=neff_name)

    patched._wait_split = True
    bass2jax.compile_bir_kernel = patched


def _install_ntff_hook():
    """Register the ctypes NTFF profile hook missing from this image's
    antenv, so run_bass_kernel_spmd(trace=True) can report exec time."""
    import contextlib
    import ctypes
    import sys
    import types

    if "antenv.axon_hooks" in sys.modules:
        return

    so_path = "/opt/axon/libaxon_pjrt.so"
    try:
        lib = ctypes.CDLL(so_path)
    except OSError:
        return
    if not hasattr(lib, "axon_start_nrt_profile"):
        return
    lib.axon_start_nrt_profile.argtypes = [ctypes.POINTER(ctypes.c_int64),
                                           ctypes.c_size_t]
    lib.axon_start_nrt_profile.restype = ctypes.c_int64
    lib.axon_stop_nrt_profile.argtypes = [ctypes.c_char_p]
    lib.axon_stop_nrt_profile.restype = ctypes.c_int64

    @contextlib.contextmanager
    def _hook(output_dir, device_ids):
        import jax
        jax.devices()
        if device_ids:
            ids = (ctypes.c_int64 * len(device_ids))(*device_ids)
            rc = lib.axon_start_nrt_profile(ids, len(device_ids))
        else:
            rc = lib.axon_start_nrt_profile(None, 0)
        if rc != 0:
            raise RuntimeError(f"axon_start_nrt_profile rc={rc}")
        try:
            yield
        finally:
            n = lib.axon_stop_nrt_profile(str(output_dir).encode())
            print(f"ntff profile: {n} file(s) in {output_dir}", file=sys.stderr)

    mod = types.ModuleType("antenv.axon_hooks")
    mod.get_axon_ntff_profile_hook = lambda: _hook
    mod.set_axon_ntff_profile_hook = lambda h: None
    sys.modules["antenv.axon_hooks"] = mod
    import antenv
    antenv.axon_hooks = mod


_module_cache = {}


def _build_module(with_mask):
    import concourse.bass as bass
    import concourse.mybir as mybir
    import concourse.tile as tile
    from concourse.masks import make_identity
    from collections import deque
    from contextlib import ExitStack

    f32 = mybir.dt.float32
    bf16 = mybir.dt.bfloat16
    u16 = mybir.dt.uint16
    Exp = mybir.ActivationFunctionType.Exp

    nc = bass.Bass("TRN2", target_bir_lowering=False)
    q_s = nc.dram_tensor("q_s", [NP, S, D], f32, kind="ExternalInput")
    kv_s = nc.dram_tensor("kv_s", [NP, S, D], f32, kind="ExternalInput")
    out_s = nc.dram_tensor("out_s", [NP, S, D], f32, kind="ExternalOutput")
    mask_t = None
    if with_mask:
        mask_t = nc.dram_tensor("mask_t", [S, S], f32, kind="ExternalInput")

    with tile.TileContext(nc) as tc, ExitStack() as ctx:
        io = ctx.enter_context(tc.tile_pool(name="io", bufs=2))
        kvp = ctx.enter_context(tc.tile_pool(name="kvp", bufs=3))
        tduo = ctx.enter_context(tc.tile_pool(name="tduo", bufs=2))
        big = ctx.enter_context(tc.tile_pool(name="big", bufs=2))
        outp = ctx.enter_context(tc.tile_pool(name="outp", bufs=2))
        res = ctx.enter_context(tc.tile_pool(name="res", bufs=3))
        cons = ctx.enter_context(tc.tile_pool(name="cons", bufs=1))
        dram = ctx.enter_context(tc.tile_pool(name="dram", bufs=2, space="DRAM"))
        # PSUM budget (8 banks): 3 x [128, 1024] score tiles (6 banks,
        # triple-buffered so the PE runs 1.5 mm1 calls ahead of the exp
        # drain - anything less re-engages the HAM clock gate) + a
        # 2-slot pool (1 bank each) shared by the mm2 accumulator and
        # the finalize transpose groups.
        ps_score = ctx.enter_context(tc.tile_pool(name="ps_score", bufs=3, space="PSUM"))
        ps_mask = (ctx.enter_context(tc.tile_pool(name="ps_mask", bufs=2))
                   if with_mask else None)
        ps_small = ctx.enter_context(tc.tile_pool(name="ps_small", bufs=2, space="PSUM"))

        identity = cons.tile([KCOLS, KCOLS], f32, tag="identity", name="identity")
        make_identity(nc, identity)

        # Warmup burst: junk matmuls queued while the prologue DMAs are
        # in flight keep the PE array busy so the HAM clock-gate
        # releases (1.2 -> 2.4 GHz) before the first real matmul.
        junk = cons.tile([P, 512], bf16, tag="junk", name="junk")
        nc.vector.memset(junk[:], 0.5)
        wtile = ps_small.tile([KCOLS, 512], f32, tag="pst", name="warm")
        for _ in range(WARMUP):
            nc.tensor.matmul(wtile[:, 0:512][:KCOLS], lhsT=junk[:, 0:KCOLS],
                             rhs=junk[:], start=True, stop=True)

        state = [dict() for _ in range(NP)]

        def prep_alloc(p):
            qf = io.tile([P, SK_BLKS, D], f32, tag="qf", name="qf")
            kf = io.tile([P, SK_BLKS, D], f32, tag="kf", name="kf")
            qb = io.tile([P, SK_BLKS, D], bf16, tag="qb", name="qb")
            kv_aug = kvp.tile([P, SK_BLKS, KCOLS], bf16, tag="kv_aug", name="kv_aug")
            qT = tduo.tile([P, S], bf16, tag="qT", name="qT")
            # kvT columns hold sk block j (the strided row set
            # {cc*16+j}, consistent with kv_aug's o index) at
            # [128j, 128j+128): mm1 reads contiguous column blocks.
            kvT = tduo.tile([P, S], bf16, tag="kvT", name="kvT")
            scr_q = dram.tile([S, P], bf16, tag="scr_q", name="scr_q")
            scr_kv = dram.tile([S, P], bf16, tag="scr_kv", name="scr_kv")
            state[p].update(qf=qf, kf=kf, qb=qb, kv_aug=kv_aug,
                            qT=qT, kvT=kvT, scr_q=scr_q, scr_kv=scr_kv)

        def prep_ins(p, dma_q=None, dma_kv=None):
            # Inbound fp32 DMAs. Row r of q/kv lands at partition r//16,
            # free index r%16: 16 consecutive rows = 4KB contiguous per
            # partition -> 128 fat descriptors, ~1.5us. For kv this
            # redefines sk block k as the STRIDED row set {c*16+k}; mm1
            # (via scr_kv's row order) and mm2 (via kv_aug's o index)
            # agree on the decomposition, and softmax + the mm2
            # reduction are order-agnostic in sk. q keeps the identity
            # row order (sq indexes the output).
            st = state[p]
            (dma_q or nc.sync).dma_start(
                st["qf"][:], q_s[p].rearrange("(pp o) d -> pp o d", o=SK_BLKS))
            (dma_kv or nc.sync).dma_start(
                st["kf"][:], kv_s[p].rearrange("(pp o) d -> pp o d", o=SK_BLKS))

        def prep_cast(p, q_cast, kv_cast):
            # kv's bf16 goes straight into kv_aug's strided columns;
            # the scratch writes read from there.
            st = state[p]
            kv_cast.tensor_copy(out=st["kv_aug"][:, :, 0:D], in_=st["kf"][:])
            kv_cast.memset(st["kv_aug"][:, :, D:KCOLS], 1.0)
            q_cast.tensor_copy(out=st["qb"][:], in_=st["qf"][:])

        def prep_scr(p, which, c, dma=None):
            # Stage one tensor: the bf16 copy is written TWICE into the
            # row-duplicated [2, S, 64] DRAM scratch - both writes are
            # fat 2KB-per-partition descriptors - then each copy is
            # XBAR-transposed (2-byte dtype) into one 64-partition half
            # of qT/kvT, so mm1 runs two k-steps concurrently in the two
            # PE row-group halves. Scratch row r holds q/kv row r in
            # (pp o) order: identity for q (sq order preserved); for kv
            # the transposed column order cc*16+j IS the strided block
            # decomposition that kv_aug/mm2 use. A transpose follows its
            # own scratch writes on the SAME queue, so it never
            # head-blocks another chain stage.
            st = state[p]
            dma = dma or nc.sync
            if which == "q":
                # scr_q row pp*16+o <- qb[pp, o]: identity row order;
                # chunk c = partitions [64c, 64c+64) = scratch rows /
                # qT cols [1024c, +1024). Two writes (the duplicated
                # 64-col halves) keep descriptors small - DMA reads of
                # one SBUF partition serialize, so fatter is SLOWER.
                scr_v = st["scr_q"].rearrange(
                    "(pp o) (u dd) -> pp o u dd", o=SK_BLKS, dd=D)
                for u in (0, 1):
                    dma.dma_start(scr_v[64 * c:64 * c + 64, :, u, :],
                                  st["qb"][64 * c:64 * c + 64, :, :])
                dma.dma_start_transpose(st["qT"][:, c * 1024:(c + 1) * 1024],
                                        st["scr_q"][c * 1024:(c + 1) * 1024, :])
            else:
                # scr_kv row j*128+cc <- kv_aug[cc, j] = kv row cc*16+j:
                # sk block j (the strided row set {cc*16+j}) lands in
                # contiguous scratch rows [128j, 128j+128) = the kvT
                # column block mm1 reads; chunk c = blocks [8c, 8c+8).
                scr_v = st["scr_kv"].rearrange(
                    "(o pp) (u dd) -> pp o u dd", pp=P, dd=D)
                for u in (0, 1):
                    dma.dma_start(scr_v[:, 8 * c:8 * c + 8, u, :],
                                  st["kv_aug"][:, 8 * c:8 * c + 8, 0:D])
                dma.dma_start_transpose(st["kvT"][:, c * 1024:(c + 1) * 1024],
                                        st["scr_kv"][c * 1024:(c + 1) * 1024, :])

        def mm1_half(p, ip, half, slot, fillers=0):
            # scoreT [128 sk x 1024 sq] for TWO sk blocks 2*ip and 2*ip+1,
            # run concurrently in PE row groups 0-63 / 64-127.
            st = state[p]
            scs = []
            for mb in (0, 1):
                i = 2 * ip + mb
                h0 = D * mb
                sc = ps_score.tile([P, HB], f32, tag="sc", name="sc")
                scs.append((i, h0, sc))
            for _ in range(fillers):
                for (i, h0, sc) in scs:
                    nc.tensor.matmul(
                        sc[:, 0:512],
                        lhsT=st["kvT"][h0:h0 + D, i * P:(i + 1) * P],
                        rhs=st["qT"][h0:h0 + D, 0:512],
                        start=True, stop=True)
            # n-major so adjacent instructions target the two PE row
            # groups and execute concurrently (mb-major serializes: the
            # in-order dispatch blocks mb1 behind mb0's second matmul).
            for n in range(HB // 512):
                c0 = half * HB + n * 512
                for (i, h0, sc) in scs:
                    nc.tensor.matmul(
                        sc[:, n * 512:(n + 1) * 512],
                        lhsT=st["kvT"][h0:h0 + D, i * P:(i + 1) * P],
                        rhs=st["qT"][h0:h0 + D, c0:c0 + 512],
                        start=True, stop=True)
            for idx, (i, h0, sc) in enumerate(scs):
                at = st["attnT"][:, i, half * HB:(half + 1) * HB]
                if with_mask:
                    mt = ps_mask.tile([P, HB], f32, tag="mt", name="mt")
                    nc.sync.dma_start(mt[:], mask_t[i * P:(i + 1) * P,
                                                    half * HB:(half + 1) * HB])
                    nc.vector.scalar_tensor_tensor(
                        out=sc[:], in0=sc[:], scalar=0.125, in1=mt[:],
                        op0=mybir.AluOpType.mult, op1=mybir.AluOpType.add)
                    nc.scalar.activation(at, sc[:], Exp)
                elif (2 * slot + idx) in _DVE_SLOTS:
                    # One-instruction exp: fp32 w = sc*C1 + C2 converts
                    # to uint16 (round-to-nearest) == bf16 exp bits.
                    nc.vector.tensor_scalar(
                        out=at.bitcast(u16), in0=sc[:],
                        scalar1=SCH_C1, scalar2=SCH_C2,
                        op0=mybir.AluOpType.mult, op1=mybir.AluOpType.add)
                else:
                    # exp((q @ kv^T) * 0.125): the 1/sqrt(D) folds into
                    # the activation's free affine scale.
                    nc.scalar.activation(at, sc[:], Exp, scale=0.125)

        def mm2_subchunk(p, n, k0, po):
            # Continue outT[0:65, n*512:(n+1)*512] over sk blocks k0..k0+3.
            st = state[p]
            for k in range(k0, k0 + 4):
                nc.tensor.matmul(
                    po[:],
                    lhsT=st["kv_aug"][:, k, :],
                    rhs=st["attnT"][:, k, n * 512:(n + 1) * 512],
                    start=(k == 0), stop=(k == SK_BLKS - 1))
            if k0 + 4 == SK_BLKS:
                nc.vector.tensor_copy(
                    out=st["outT"][:, n * 512:(n + 1) * 512], in_=po[:])

        def finalize_g(p, g):
            # Transpose 128-column blocks 4g..4g+3 back to [sq, d] in one
            # PSUM group (borrowing a score-ring slot), normalize with
            # one reciprocal + one broadcast multiply, store 512 rows
            # with one DMA.
            st = state[p]
            tp4 = ps_small.tile([P, 4, KCOLS], f32, tag="pst", name="tp4")
            for jj in range(4):
                j = 4 * g + jj
                nc.tensor.transpose(tp4[:, jj, :],
                                    st["outT"][:, j * P:(j + 1) * P], identity[:])
            rec4 = res.tile([P, 4], f32, tag="rec4", name="rec4")
            nc.vector.reciprocal(rec4[:], tp4[:, :, D])
            ob4 = res.tile([P, 4, D], f32, tag="ob4", name="ob4")
            nc.vector.tensor_tensor(
                out=ob4[:], in0=tp4[:, :, 0:D],
                in1=rec4[:, :, None].broadcast_to([P, 4, D]),
                op=mybir.AluOpType.mult)
            nc.sync.dma_start(
                out_s[p, g * 512:(g + 1) * 512, :].rearrange(
                    "(j pp) d -> pp j d", pp=P),
                ob4[:])

        sub_q = deque()    # (pair, n, k0)
        fins_q = deque()   # (pair, g)
        chunks_done = [0] * NP
        cur_po = [None]

        def pop_sub():
            if sub_q:
                p, n, k0 = sub_q.popleft()
                if k0 == 0:
                    cur_po[0] = ps_small.tile([KCOLS, 512], f32, tag="pst", name="po")
                mm2_subchunk(p, n, k0, cur_po[0])
                if k0 + 4 == SK_BLKS:
                    chunks_done[p] += 1

        def pop_fin():
            if fins_q:
                p, g = fins_q[0]
                if g < chunks_done[p]:
                    fins_q.popleft()
                    finalize_g(p, g)

        # Pair 0's prep is chunked x4 so the first mm1 can start as soon
        # as the first chunk's transposes land instead of waiting for
        # the full chain.
        # Pair 0's prep is latency-critical: q's whole chain rides the
        # otherwise-idle Scalar HWDGE queue in parallel with kv's on
        # Sync, and casts go to VectorE (idle in the prologue, 4x
        # faster than GpSimdE).
        prep_alloc(0)
        prep_ins(0, dma_q=nc.scalar, dma_kv=nc.sync)
        prep_cast(0, q_cast=nc.vector, kv_cast=nc.vector)
        # First-needed chunks split across the queues; each queue then
        # carries one second chunk, so everything lands by ~slot 4.
        prep_scr(0, "q", 0, dma=nc.scalar)
        prep_scr(0, "k", 0, dma=nc.sync)
        prep_scr(0, "k", 1, dma=nc.sync)
        prep_scr(0, "q", 1, dma=nc.sync)
        for p in range(NP):
            state[p]["attnT"] = big.tile([P, SK_BLKS, S], bf16, tag="attnT", name="attnT")
            state[p]["outT"] = outp.tile([KCOLS, S], f32, tag="outT", name="outT")
            slot = 0
            for half in range(S // HB):
                for ip in range(SK_BLKS // 2):
                    # Emit the independent backlog first so the PE stream
                    # never has a dependent mm1 at its head while older
                    # work could run. Fins pop before subs so a fin's
                    # transposes trail its outT copies by a full slot.
                    pop_fin()
                    pop_sub()
                    # Pair 0's early slots have no mm2/fin backlog to
                    # absorb the exp-paced ramp; fillers writing the
                    # about-to-be-overwritten score slices keep the PE
                    # dense so the HAM clock-gate stays released.
                    mm1_half(p, ip, half, slot,
                             fillers=2 if p == 0 and slot < 6 else 0)
                    slot += 1
                    # Next pair's prep, emitted early with casts on the
                    # idle GpSimdE (kv first: mm1 walks all kv blocks
                    # within its first half) and all DMAs on Sync, so
                    # the ~12us chain lands well before that pair's mm1.
                    if p + 1 < NP:
                        if slot == 1:
                            prep_alloc(p + 1)
                            prep_ins(p + 1)
                            prep_cast(p + 1, q_cast=nc.gpsimd,
                                      kv_cast=nc.gpsimd)
                        elif slot == 3:
                            prep_scr(p + 1, "k", 0)
                            prep_scr(p + 1, "q", 0)
                        elif slot == 5:
                            prep_scr(p + 1, "k", 1)
                            prep_scr(p + 1, "q", 1)
                # This half's sq cols now have all 16 sk blocks: queue mm2.
                for n in (2 * half, 2 * half + 1):
                    for k0 in range(0, SK_BLKS, 4):
                        sub_q.append((p, n, k0))
                fins_q.append((p, 2 * half))
                fins_q.append((p, 2 * half + 1))
        while sub_q or fins_q:
            pop_sub()
            pop_fin()

    return nc


def _get_module(with_mask):
    if with_mask not in _module_cache:
        _install_wait_split()
        _install_ntff_hook()
        _module_cache[with_mask] = _build_module(with_mask)
    return _module_cache[with_mask]


def _run(q, kv, mask, trace=False, tmpdir=None):
    from concourse.bass_utils import run_bass_kernel_spmd

    q = np.ascontiguousarray(np.asarray(q), dtype=np.float32)
    kv = np.ascontiguousarray(np.asarray(kv), dtype=np.float32)
    mask = np.asarray(mask)
    with_mask = bool(np.any(mask))

    nc = _get_module(with_mask)

    qf = q.reshape(B * H, S, D)
    kf = kv.reshape(B * H, S, D)
    in_maps = []
    for c in range(N_CORES):
        m = {
            "q_s": np.ascontiguousarray(qf[c * NP:(c + 1) * NP]),
            "kv_s": np.ascontiguousarray(kf[c * NP:(c + 1) * NP]),
        }
        if with_mask:
            m["mask_t"] = np.ascontiguousarray(
                mask.reshape(S, S).T, dtype=np.float32)
        in_maps.append(m)

    kw = {}
    if trace:
        kw = dict(trace=True, tmpdir=tmpdir)
    bres = run_bass_kernel_spmd(nc, in_maps, core_ids=list(range(N_CORES)), **kw)
    out = np.stack([bres.results[c]["out_s"] for c in range(N_CORES)])
    out = out.reshape(B, H, S, D).astype(np.float32, copy=False)
    return out, bres


def kernel(q, kv, mask):
    out, _ = _run(q, kv, mask)
    return out


# revision 73
# speedup vs baseline: 1.1309x; 1.0326x over previous
"""Trainium2 Bass kernel for batched dot-product attention.

Problem: q, kv [B=4, H=8, S=2048, D=64] fp32, mask [1, 1, S, S] fp32.
    out = softmax(q @ kv^T / sqrt(D) + mask) @ kv

Sharding: the 32 (b, h) pairs are split across 8 NeuronCores, 4 pairs
per core. Each core computes its pairs' full S x S attention locally;
no cross-device communication.

Per-pair device algorithm (fast path, mask == 0):
  1. q, kv are DMA'd in fp32 with fat per-partition descriptors (row r
     at partition r//16), cast to bf16 (GpSimdE steady-state / VectorE
     for the latency-critical pair 0), staged to a DRAM scratch
     [S, 128] with the 64 columns DUPLICATED into both halves via two
     small-descriptor DMA writes (XBAR transpose needs a 2-byte dtype;
     fat single-partition descriptors would serialize on one SBUF
     port), and DMA-transposed back into qT/kvT [128, S] bf16 tiles
     holding the transposed tensor in BOTH partition ranges 0-63 and
     64-127. kv's sk block k is the STRIDED row set {c*16+k} - mm1
     (via scratch row order) and mm2 (via kv_aug's o index) agree, and
     softmax + the mm2 reduction are order-agnostic in sk. The pair-0
     chain is chunked x2 and split across the Sync and Scalar HWDGE
     queues so the first matmul can start at ~13us.
  2. scoreT[sk, sq] = kvT.T @ qT per 128-row sk block into PSUM: the
     duplicated halves let two K=64 matmuls (sk blocks 2i, 2i+1) run
     CONCURRENTLY in the two PE row-group halves. The exp drain is
     SPLIT between two engines, tile by tile:
       - ScalarE: activation exp(0.125 * scoreT) -> bf16 attnT.
       - VectorE: a Schraudolph-style one-instruction exp. tensor_scalar
         computes w = scoreT*C1 + C2 in fp32 and converts to uint16 with
         round-to-nearest; C1/C2 are chosen so that integer IS the bf16
         bit pattern of exp(0.125*scoreT) (max mult. error +-3.4%,
         which washes to ~4e-3 rel error after softmax normalization).
     Splitting 19:13 keeps both engines' exp streams at ~83us, below
     the PE's ~128us, so the kernel is TensorE-bound (~90% PE busy).
     Softmax max-subtraction is skipped: scores are ~N(0,64) pre-scale
     so exp is safe in fp32, matching the reference to ~4e-3.
  3. outT[d, sq] (+ a denominator row) = kv_aug.T @ attnT accumulated
     over the 16 sk blocks, where kv_aug [128, 16, 65] bf16 is kv with
     a ones column: row 64 of outT is the softmax denominator. The cast
     writes kv's bf16 directly into kv_aug's strided columns.
  4. outT 128-column blocks are transposed back on TensorE (identity
     matmul) in groups of 4 into one PSUM tile [128, 4, 65]; one
     VectorE reciprocal + one broadcast multiply normalize the group,
     and one DMA stores 512 rows of fp32 output.

mm1 iterates half-outer (all 16 sk blocks for sq cols [0,1024) before
cols [1024,2048)), so each pair's mm2 n-tiles 0-1 start while mm1 still
runs on the second half, shrinking the pipeline tail. A warmup burst of
junk matmuls during the prologue DMAs keeps the PE HAM clock-gate
released (1.2 -> 2.4 GHz) before the first real matmul; pair-0's early
slots add score-slice fillers so the exp-paced ramp never idles the PE
long enough to re-engage the gate. Engine-queue discipline matters
everywhere: an instruction that waits at the head of an in-order queue
(HWDGE Sync/Scalar, or the PE) blocks everything behind it, so work is
emitted so that queue order matches data-readiness order.

If mask is nonzero (never the case for this problem's setup_inputs,
which zero-fills it), a variant NEFF streams mask^T tiles and adds them
to scoreT before a ScalarE-only exp. Slower, but correct.
"""

import math

import numpy as np

B, H, S, D = 4, 8, 2048, 64
N_CORES = 8
NP = (B * H) // N_CORES  # pairs per core = 4
P = 128
SK_BLKS = S // P   # 16
NT = S // 512      # 4 sq tiles of 512
KCOLS = D + 1      # kv columns + ones column
HB = 1024          # score tile free size (2 PSUM banks)
WARMUP = 48        # prologue junk matmuls: sized to bridge the PE from
                   # engine start to first mm1 (~25us) so the HAM
                   # clock-gate never re-engages

# Schraudolph constants: uint16(round(score*C1 + C2)) == bf16 bits of
# ~exp(score/8).  C1 = 128*log2(e)/8, C2 = 128*(127 - c_opt).
SCH_C1 = float(16.0 * math.log2(math.e))
SCH_C2 = float(128.0 * (127.0 - 0.0380))

# Of the 32 exp tiles per pair, this many go to VectorE (Bresenham-
# interleaved with ScalarE tiles so the two PSUM drains overlap).
DVE_N = 13         # of 32 exp tiles per pair, this many go to VectorE
_DVE_SLOTS = frozenset(
    s for s in range(32) if (s * DVE_N) // 32 != ((s + 1) * DVE_N) // 32)


def _install_wait_split():
    """Split multi-sem-wait instructions into single-wait NoOp carriers.

    The walrus build in this container rejects any instruction whose
    sync_info.on_wait has more than one entry ("Too many sync wait
    commands"). Engines execute their stream in order, so hoisting all
    but one wait onto same-engine NoOps directly before the instruction
    is semantically identical.
    """
    import orjson
    import concourse.bass2jax as bass2jax
    import concourse.bass_utils as bass_utils

    if getattr(bass2jax.compile_bir_kernel, "_wait_split", False):
        return

    def split_multi_waits(bir_json):
        d = orjson.loads(bir_json)
        for fn in d.get("functions", []):
            for blk in fn.get("blocks", []):
                out = []
                for inst in blk.get("instructions", []):
                    si = inst.get("sync_info") or {}
                    ow = si.get("on_wait") or []
                    if len(ow) > 1:
                        for j, w in enumerate(ow[:-1]):
                            out.append({
                                "engine": inst["engine"],
                                "ins": [],
                                "name": f"{inst['name']}-w{j}",
                                "opcode": "NoOp",
                                "outs": [],
                                "sync_info": {"on_wait": [w]},
                            })
                        si["on_wait"] = [ow[-1]]
                    out.append(inst)
                blk["instructions"] = out
        return orjson.dumps(d)

    orig = bass_utils.compile_bir_kernel

    def patched(bir_json, tmpdir, neff_name="file.neff"):
        return orig(split_multi_waits(bir_json), tmpdir, neff_name=neff_name)

    patched._wait_split = True
    bass2jax.compile_bir_kernel = patched


def _install_ntff_hook():
    """Register the ctypes NTFF profile hook missing from this image's
    antenv, so run_bass_kernel_spmd(trace=True) can report exec time."""
    import contextlib
    import ctypes
    import sys
    import types

    if "antenv.axon_hooks" in sys.modules:
        return

    so_path = "/opt/axon/libaxon_pjrt.so"
    try:
        lib = ctypes.CDLL(so_path)
    except OSError:
        return
    if not hasattr(lib, "axon_start_nrt_profile"):
        return
    lib.axon_start_nrt_profile.argtypes = [ctypes.POINTER(ctypes.c_int64),
                                           ctypes.c_size_t]
    lib.axon_start_nrt_profile.restype = ctypes.c_int64
    lib.axon_stop_nrt_profile.argtypes = [ctypes.c_char_p]
    lib.axon_stop_nrt_profile.restype = ctypes.c_int64

    @contextlib.contextmanager
    def _hook(output_dir, device_ids):
        import jax
        jax.devices()
        if device_ids:
            ids = (ctypes.c_int64 * len(device_ids))(*device_ids)
            rc = lib.axon_start_nrt_profile(ids, len(device_ids))
        else:
            rc = lib.axon_start_nrt_profile(None, 0)
        if rc != 0:
            raise RuntimeError(f"axon_start_nrt_profile rc={rc}")
        try:
            yield
        finally:
            n = lib.axon_stop_nrt_profile(str(output_dir).encode())
            print(f"ntff profile: {n} file(s) in {output_dir}", file=sys.stderr)

    mod = types.ModuleType("antenv.axon_hooks")
    mod.get_axon_ntff_profile_hook = lambda: _hook
    mod.set_axon_ntff_profile_hook = lambda h: None
    sys.modules["antenv.axon_hooks"] = mod
    import antenv
    antenv.axon_hooks = mod


_module_cache = {}


def _build_module(with_mask):
    import concourse.bass as bass
    import concourse.mybir as mybir
    import concourse.tile as tile
    from concourse.masks import make_identity
    from collections import deque
    from contextlib import ExitStack

    f32 = mybir.dt.float32
    bf16 = mybir.dt.bfloat16
    u16 = mybir.dt.uint16
    Exp = mybir.ActivationFunctionType.Exp

    nc = bass.Bass("TRN2", target_bir_lowering=False)
    q_s = nc.dram_tensor("q_s", [NP, S, D], f32, kind="ExternalInput")
    kv_s = nc.dram_tensor("kv_s", [NP, S, D], f32, kind="ExternalInput")
    out_s = nc.dram_tensor("out_s", [NP, S, D], f32, kind="ExternalOutput")
    mask_t = None
    if with_mask:
        mask_t = nc.dram_tensor("mask_t", [S, S], f32, kind="ExternalInput")

    with tile.TileContext(nc) as tc, ExitStack() as ctx:
        io = ctx.enter_context(tc.tile_pool(name="io", bufs=2))
        kvp = ctx.enter_context(tc.tile_pool(name="kvp", bufs=3))
        tduo = ctx.enter_context(tc.tile_pool(name="tduo", bufs=2))
        big = ctx.enter_context(tc.tile_pool(name="big", bufs=2))
        outp = ctx.enter_context(tc.tile_pool(name="outp", bufs=2))
        res = ctx.enter_context(tc.tile_pool(name="res", bufs=3))
        cons = ctx.enter_context(tc.tile_pool(name="cons", bufs=1))
        dram = ctx.enter_context(tc.tile_pool(name="dram", bufs=2, space="DRAM"))
        # PSUM budget (8 banks): 3 x [128, 1024] score tiles (6 banks,
        # triple-buffered so the PE runs 1.5 mm1 calls ahead of the exp
        # drain - anything less re-engages the HAM clock gate) + a
        # 2-slot pool (1 bank each) shared by the mm2 accumulator and
        # the finalize transpose groups.
        ps_score = ctx.enter_context(tc.tile_pool(name="ps_score", bufs=3, space="PSUM"))
        ps_mask = (ctx.enter_context(tc.tile_pool(name="ps_mask", bufs=2))
                   if with_mask else None)
        ps_small = ctx.enter_context(tc.tile_pool(name="ps_small", bufs=2, space="PSUM"))

        identity = cons.tile([KCOLS, KCOLS], f32, tag="identity", name="identity")
        make_identity(nc, identity)

        # Warmup burst: junk matmuls queued while the prologue DMAs are
        # in flight keep the PE array busy so the HAM clock-gate
        # releases (1.2 -> 2.4 GHz) before the first real matmul.
        junk = cons.tile([P, 512], bf16, tag="junk", name="junk")
        nc.vector.memset(junk[:], 0.5)
        wtile = ps_small.tile([KCOLS, 512], f32, tag="pst", name="warm")
        for _ in range(WARMUP):
            nc.tensor.matmul(wtile[:, 0:512][:KCOLS], lhsT=junk[:, 0:KCOLS],
                             rhs=junk[:], start=True, stop=True)

        state = [dict() for _ in range(NP)]

        def prep_alloc(p):
            qf = io.tile([P, SK_BLKS, D], f32, tag="qf", name="qf")
            kf = io.tile([P, SK_BLKS, D], f32, tag="kf", name="kf")
            qb = io.tile([P, SK_BLKS, D], bf16, tag="qb", name="qb")
            kv_aug = kvp.tile([P, SK_BLKS, KCOLS], bf16, tag="kv_aug", name="kv_aug")
            qT = tduo.tile([P, S], bf16, tag="qT", name="qT")
            # kvT columns hold sk block j (the strided row set
            # {cc*16+j}, consistent with kv_aug's o index) at
            # [128j, 128j+128): mm1 reads contiguous column blocks.
            kvT = tduo.tile([P, S], bf16, tag="kvT", name="kvT")
            scr_q = dram.tile([S, P], bf16, tag="scr_q", name="scr_q")
            scr_kv = dram.tile([S, P], bf16, tag="scr_kv", name="scr_kv")
            state[p].update(qf=qf, kf=kf, qb=qb, kv_aug=kv_aug,
                            qT=qT, kvT=kvT, scr_q=scr_q, scr_kv=scr_kv)

        def prep_ins(p, dma_q=None, dma_kv=None):
            # Inbound fp32 DMAs. Row r of q/kv lands at partition r//16,
            # free index r%16: 16 consecutive rows = 4KB contiguous per
            # partition -> 128 fat descriptors, ~1.5us. For kv this
            # redefines sk block k as the STRIDED row set {c*16+k}; mm1
            # (via scr_kv's row order) and mm2 (via kv_aug's o index)
            # agree on the decomposition, and softmax + the mm2
            # reduction are order-agnostic in sk. q keeps the identity
            # row order (sq indexes the output).
            st = state[p]
            (dma_q or nc.sync).dma_start(
                st["qf"][:], q_s[p].rearrange("(pp o) d -> pp o d", o=SK_BLKS))
            (dma_kv or nc.sync).dma_start(
                st["kf"][:], kv_s[p].rearrange("(pp o) d -> pp o d", o=SK_BLKS))

        def prep_cast(p, q_cast, kv_cast):
            # kv's bf16 goes straight into kv_aug's strided columns;
            # the scratch writes read from there.
            st = state[p]
            kv_cast.tensor_copy(out=st["kv_aug"][:, :, 0:D], in_=st["kf"][:])
            kv_cast.memset(st["kv_aug"][:, :, D:KCOLS], 1.0)
            q_cast.tensor_copy(out=st["qb"][:], in_=st["qf"][:])

        def prep_scr(p, which, c, dma=None):
            # Stage one tensor: the bf16 copy is written TWICE into the
            # row-duplicated [2, S, 64] DRAM scratch - both writes are
            # fat 2KB-per-partition descriptors - then each copy is
            # XBAR-transposed (2-byte dtype) into one 64-partition half
            # of qT/kvT, so mm1 runs two k-steps concurrently in the two
            # PE row-group halves. Scratch row r holds q/kv row r in
            # (pp o) order: identity for q (sq order preserved); for kv
            # the transposed column order cc*16+j IS the strided block
            # decomposition that kv_aug/mm2 use. A transpose follows its
            # own scratch writes on the SAME queue, so it never
            # head-blocks another chain stage.
            st = state[p]
            dma = dma or nc.sync
            if which == "q":
                # scr_q row pp*16+o <- qb[pp, o]: identity row order;
                # chunk c = partitions [64c, 64c+64) = scratch rows /
                # qT cols [1024c, +1024). Two writes (the duplicated
                # 64-col halves) keep descriptors small - DMA reads of
                # one SBUF partition serialize, so fatter is SLOWER.
                scr_v = st["scr_q"].rearrange(
                    "(pp o) (u dd) -> pp o u dd", o=SK_BLKS, dd=D)
                for u in (0, 1):
                    dma.dma_start(scr_v[64 * c:64 * c + 64, :, u, :],
                                  st["qb"][64 * c:64 * c + 64, :, :])
                dma.dma_start_transpose(st["qT"][:, c * 1024:(c + 1) * 1024],
                                        st["scr_q"][c * 1024:(c + 1) * 1024, :])
            else:
                # scr_kv row j*128+cc <- kv_aug[cc, j] = kv row cc*16+j:
                # sk block j (the strided row set {cc*16+j}) lands in
                # contiguous scratch rows [128j, 128j+128) = the kvT
                # column block mm1 reads; chunk c = blocks [8c, 8c+8).
                scr_v = st["scr_kv"].rearrange(
                    "(o pp) (u dd) -> pp o u dd", pp=P, dd=D)
                for u in (0, 1):
                    dma.dma_start(scr_v[:, 8 * c:8 * c + 8, u, :],
                                  st["kv_aug"][:, 8 * c:8 * c + 8, 0:D])
                dma.dma_start_transpose(st["kvT"][:, c * 1024:(c + 1) * 1024],
                                        st["scr_kv"][c * 1024:(c + 1) * 1024, :])

        def mm1_half(p, ip, half, slot, fillers=0):
            # scoreT [128 sk x 1024 sq] for TWO sk blocks 2*ip and 2*ip+1,
            # run concurrently in PE row groups 0-63 / 64-127.
            st = state[p]
            scs = []
            for mb in (0, 1):
                i = 2 * ip + mb
                h0 = D * mb
                sc = ps_score.tile([P, HB], f32, tag="sc", name="sc")
                scs.append((i, h0, sc))
            for _ in range(fillers):
                for (i, h0, sc) in scs:
                    nc.tensor.matmul(
                        sc[:, 0:512],
                        lhsT=st["kvT"][h0:h0 + D, i * P:(i + 1) * P],
                        rhs=st["qT"][h0:h0 + D, 0:512],
                        start=True, stop=True)
            # n-major so adjacent instructions target the two PE row
            # groups and execute concurrently (mb-major serializes: the
            # in-order dispatch blocks mb1 behind mb0's second matmul).
            for n in range(HB // 512):
                c0 = half * HB + n * 512
                for (i, h0, sc) in scs:
                    nc.tensor.matmul(
                        sc[:, n * 512:(n + 1) * 512],
                        lhsT=st["kvT"][h0:h0 + D, i * P:(i + 1) * P],
                        rhs=st["qT"][h0:h0 + D, c0:c0 + 512],
                        start=True, stop=True)
            for idx, (i, h0, sc) in enumerate(scs):
                at = st["attnT"][:, i, half * HB:(half + 1) * HB]
                if with_mask:
                    mt = ps_mask.tile([P, HB], f32, tag="mt", name="mt")
                    nc.sync.dma_start(mt[:], mask_t[i * P:(i + 1) * P,
                                                    half * HB:(half + 1) * HB])
                    nc.vector.scalar_tensor_tensor(
                        out=sc[:], in0=sc[:], scalar=0.125, in1=mt[:],
                        op0=mybir.AluOpType.mult, op1=mybir.AluOpType.add)
                    nc.scalar.activation(at, sc[:], Exp)
                elif (2 * slot + idx) in _DVE_SLOTS:
                    # One-instruction exp: fp32 w = sc*C1 + C2 converts
                    # to uint16 (round-to-nearest) == bf16 exp bits.
                    nc.vector.tensor_scalar(
                        out=at.bitcast(u16), in0=sc[:],
                        scalar1=SCH_C1, scalar2=SCH_C2,
                        op0=mybir.AluOpType.mult, op1=mybir.AluOpType.add)
                else:
                    # exp((q @ kv^T) * 0.125): the 1/sqrt(D) folds into
                    # the activation's free affine scale.
                    nc.scalar.activation(at, sc[:], Exp, scale=0.125)

        def mm2_subchunk(p, n, k0, po):
            # Continue outT[0:65, n*512:(n+1)*512] over sk blocks k0..k0+3.
            st = state[p]
            for k in range(k0, k0 + 4):
                nc.tensor.matmul(
                    po[:],
                    lhsT=st["kv_aug"][:, k, :],
                    rhs=st["attnT"][:, k, n * 512:(n + 1) * 512],
                    start=(k == 0), stop=(k == SK_BLKS - 1))
            if k0 + 4 == SK_BLKS:
                nc.vector.tensor_copy(
                    out=st["outT"][:, n * 512:(n + 1) * 512], in_=po[:])

        def finalize_g(p, g):
            # Transpose 128-column blocks 4g..4g+3 back to [sq, d] in one
            # PSUM group (borrowing a score-ring slot), normalize with
            # one reciprocal + one broadcast multiply, store 512 rows
            # with one DMA.
            st = state[p]
            tp4 = ps_small.tile([P, 4, KCOLS], f32, tag="pst", name="tp4")
            for jj in range(4):
                j = 4 * g + jj
                nc.tensor.transpose(tp4[:, jj, :],
                                    st["outT"][:, j * P:(j + 1) * P], identity[:])
            rec4 = res.tile([P, 4], f32, tag="rec4", name="rec4")
            nc.vector.reciprocal(rec4[:], tp4[:, :, D])
            ob4 = res.tile([P, 4, D], f32, tag="ob4", name="ob4")
            nc.vector.tensor_tensor(
                out=ob4[:], in0=tp4[:, :, 0:D],
                in1=rec4[:, :, None].broadcast_to([P, 4, D]),
                op=mybir.AluOpType.mult)
            nc.sync.dma_start(
                out_s[p, g * 512:(g + 1) * 512, :].rearrange(
                    "(j pp) d -> pp j d", pp=P),
                ob4[:])

        sub_q = deque()    # (pair, n, k0)
        fins_q = deque()   # (pair, g)
        chunks_done = [0] * NP
        cur_po = [None]

        def pop_sub():
            if sub_q:
                p, n, k0 = sub_q.popleft()
                if k0 == 0:
                    cur_po[0] = ps_small.tile([KCOLS, 512], f32, tag="pst", name="po")
                mm2_subchunk(p, n, k0, cur_po[0])
                if k0 + 4 == SK_BLKS:
                    chunks_done[p] += 1

        def pop_fin():
            if fins_q:
                p, g = fins_q[0]
                if g < chunks_done[p]:
                    fins_q.popleft()
                    finalize_g(p, g)

        # Pair 0's prep is chunked x4 so the first mm1 can start as soon
        # as the first chunk's transposes land instead of waiting for
        # the full chain.
        # Pair 0's prep is latency-critical: q's whole chain rides the
        # otherwise-idle Scalar HWDGE queue in parallel with kv's on
        # Sync, and casts go to VectorE (idle in the prologue, 4x
        # faster than GpSimdE).
        prep_alloc(0)
        prep_ins(0, dma_q=nc.scalar, dma_kv=nc.sync)
        prep_cast(0, q_cast=nc.vector, kv_cast=nc.vector)
        # First-needed chunks split across the queues; each queue then
        # carries one second chunk, so everything lands by ~slot 4.
        prep_scr(0, "q", 0, dma=nc.scalar)
        prep_scr(0, "k", 0, dma=nc.sync)
        prep_scr(0, "k", 1, dma=nc.sync)
        prep_scr(0, "q", 1, dma=nc.sync)
        for p in range(NP):
            state[p]["attnT"] = big.tile([P, SK_BLKS, S], bf16, tag="attnT", name="attnT")
            state[p]["outT"] = outp.tile([KCOLS, S], f32, tag="outT", name="outT")
            slot = 0
            for half in range(S // HB):
                for ip in range(SK_BLKS // 2):
                    # Emit the independent backlog first so the PE stream
                    # never has a dependent mm1 at its head while older
                    # work could run. Fins pop before subs so a fin's
                    # transposes trail its outT copies by a full slot.
                    pop_fin()
                    pop_sub()
                    # Pair 0's early slots have no mm2/fin backlog to
                    # absorb the exp-paced ramp; fillers writing the
                    # about-to-be-overwritten score slices keep the PE
                    # dense so the HAM clock-gate stays released.
                    mm1_half(p, ip, half, slot,
                             fillers=2 if p == 0 and slot < 6 else 0)
                    slot += 1
                    # Next pair's prep, emitted early with casts on the
                    # idle GpSimdE (kv first: mm1 walks all kv blocks
                    # within its first half) and all DMAs on Sync, so
                    # the ~12us chain lands well before that pair's mm1.
                    if p + 1 < NP:
                        if slot == 1:
                            prep_alloc(p + 1)
                            prep_ins(p + 1)
                            prep_cast(p + 1, q_cast=nc.gpsimd,
                                      kv_cast=nc.gpsimd)
                        elif slot == 3:
                            prep_scr(p + 1, "k", 0)
                            prep_scr(p + 1, "q", 0)
                        elif slot == 5:
                            prep_scr(p + 1, "k", 1)
                            prep_scr(p + 1, "q", 1)
                # This half's sq cols now have all 16 sk blocks: queue mm2.
                for n in (2 * half, 2 * half + 1):
                    for k0 in range(0, SK_BLKS, 4):
                        sub_q.append((p, n, k0))
                fins_q.append((p, 2 * half))
                fins_q.append((p, 2 * half + 1))
        while sub_q or fins_q:
            pop_sub()
            pop_fin()

    return nc


def _get_module(with_mask):
    if with_mask not in _module_cache:
        _install_wait_split()
        _install_ntff_hook()
        _module_cache[with_mask] = _build_module(with_mask)
    return _module_cache[with_mask]


def _run(q, kv, mask, trace=False, tmpdir=None):
    from concourse.bass_utils import run_bass_kernel_spmd

    q = np.ascontiguousarray(np.asarray(q), dtype=np.float32)
    kv = np.ascontiguousarray(np.asarray(kv), dtype=np.float32)
    mask = np.asarray(mask)
    with_mask = bool(np.any(mask))

    nc = _get_module(with_mask)

    qf = q.reshape(B * H, S, D)
    kf = kv.reshape(B * H, S, D)
    in_maps = []
    for c in range(N_CORES):
        m = {
            "q_s": np.ascontiguousarray(qf[c * NP:(c + 1) * NP]),
            "kv_s": np.ascontiguousarray(kf[c * NP:(c + 1) * NP]),
        }
        if with_mask:
            m["mask_t"] = np.ascontiguousarray(
                mask.reshape(S, S).T, dtype=np.float32)
        in_maps.append(m)

    kw = {}
    if trace:
        kw = dict(trace=True, tmpdir=tmpdir)
    bres = run_bass_kernel_spmd(nc, in_maps, core_ids=list(range(N_CORES)), **kw)
    out = np.stack([bres.results[c]["out_s"] for c in range(N_CORES)])
    out = out.reshape(B, H, S, D).astype(np.float32, copy=False)
    return out, bres


def kernel(q, kv, mask):
    out, _ = _run(q, kv, mask)
    return out
